# revision 1
# baseline (speedup 1.0000x reference)
"""GNN NodeBlock kernel for Trainium2, 8 NeuronCores (SPMD, no collectives).

Reference computation (N=50000 nodes, E=1600000 edges, F=128 features):
    recv_agg = segment_sum(edge_attr, edge_index[1], N)        # [N, 128]
    collected = concat([recv_agg, x, broadcast(u)], -1)        # [N, 272]
    out = relu(collected @ W1 + b1) @ W2 + b2                  # [N, 128]

Sharding: nodes are partitioned into 8 contiguous blocks of 6250; edges are
bucketed by receiver-node ownership on the host so the scatter-sum is local
to each core; MLP weights are replicated.  The u-term of layer 1 is constant
across nodes and is folded into b1 on the host (b1_eff = b1 + u @ W1[256:]).

Device algorithm per core (nodes padded to 49 tiles of 128):
  scatter: edges sorted by receiver, grouped per 128-node tile, padded to
    C_max chunks of 128 edges.  For each chunk, DVE builds a one-hot routing
    matrix onehot[e, n] = (rel[e] == n) via tensor_scalar(is_equal) against a
    constant iota tile, and the PE accumulates aggT[f, n] += edge[e, f]^T @
    onehot[e, n] into PSUM.  aggT comes out feature-major — exactly the
    layout layer 1 needs.
  L1: out1T[h, n] = W1a[d, h]^T @ aggT[d, n] + W1b[d, h]^T @ xT[d, n] over
    supertiles of up to 512 nodes, then ReLU+bias on the scalar engine.
  L2: out2[n, f] = sum_h hT[h, n]^T @ W2r[h, f], bias via a rank-1 matmul.
"""

import numpy as np

from concourse import bacc, mybir, tile
from concourse import bass_utils
from concourse.bass_interp import get_hw_module

# ---------------- problem constants (hardcoded per spec) ----------------
N_NODES = 50000
N_EDGES = 1600000
F = 128           # edge/node feature dim
H = 1024          # hidden dim
D_U = 16
N_CORES = 8
NODES_PC = N_NODES // N_CORES          # 6250
NT = (NODES_PC + 127) // 128           # 49 node tiles per core
NODES_PAD = NT * 128                   # 6272
SUPERS = [4] * (NT // 4) + ([NT % 4] if NT % 4 else [])   # [4]*12 + [1]

# dtype knobs: "edge" covers edge data + one-hot; "mlp" covers x/W1/W2 path
EDGE_DT = mybir.dt.float32
MLP_DT = mybir.dt.float32

_np = mybir.dt.np  # mybir dtype -> numpy dtype


# ---------------- device program ----------------

def build_program(c_max: int, edge_dt, mlp_dt):
    f32 = mybir.dt.float32
    nc = bacc.Bacc("TRN2", target_bir_lowering=False, debug=False,
                   num_devices=N_CORES)

    edges = nc.dram_tensor("edges", [NT, 128, c_max, F], edge_dt,
                           kind="ExternalInput").ap()
    relT = nc.dram_tensor("relT", [128, NT * c_max], edge_dt,
                          kind="ExternalInput").ap()
    iota = nc.dram_tensor("iota", [128, 128], edge_dt,
                          kind="ExternalInput").ap()
    xT = nc.dram_tensor("xT", [128, NODES_PAD], mlp_dt,
                        kind="ExternalInput").ap()
    w1a = nc.dram_tensor("w1a", [128, H], mlp_dt, kind="ExternalInput").ap()
    w1b = nc.dram_tensor("w1b", [128, H], mlp_dt, kind="ExternalInput").ap()
    w2r = nc.dram_tensor("w2r", [128, H], mlp_dt, kind="ExternalInput").ap()
    b1T = nc.dram_tensor("b1T", [128, H // 128], f32,
                         kind="ExternalInput").ap()
    bias2 = nc.dram_tensor("bias2", [1, 2 * F], mlp_dt,
                           kind="ExternalInput").ap()
    y = nc.dram_tensor("y", [NODES_PAD, F], f32, kind="ExternalOutput").ap()

    HC = H // 128  # 8 hidden chunks

    with tile.TileContext(nc) as tc:
        with (
            tc.tile_pool(name="const", bufs=1) as cpool,
            tc.tile_pool(name="edge", bufs=3) as epool,
            tc.tile_pool(name="oh", bufs=6) as ohpool,
            tc.tile_pool(name="agg", bufs=2) as aggpool,
            tc.tile_pool(name="h", bufs=2) as hpool,
            tc.tile_pool(name="out", bufs=3) as outpool,
            tc.tile_pool(name="ps_agg", bufs=2, space="PSUM") as ps_agg,
            tc.tile_pool(name="ps_h", bufs=2, space="PSUM") as ps_h,
            tc.tile_pool(name="ps_out", bufs=2, space="PSUM") as ps_out,
        ):
            # resident constants
            relT_sb = cpool.tile([128, NT * c_max], edge_dt, tag="relT")
            nc.sync.dma_start(relT_sb[:], relT[:])
            iota_sb = cpool.tile([128, 128], edge_dt, tag="iota")
            nc.sync.dma_start(iota_sb[:], iota[:])
            xT_sb = cpool.tile([128, NODES_PAD], mlp_dt, tag="xT")
            nc.sync.dma_start(xT_sb[:], xT[:])
            w1a_sb = cpool.tile([128, H], mlp_dt, tag="w1a")
            nc.sync.dma_start(w1a_sb[:], w1a[:])
            w1b_sb = cpool.tile([128, H], mlp_dt, tag="w1b")
            nc.sync.dma_start(w1b_sb[:], w1b[:])
            w2r_sb = cpool.tile([128, H], mlp_dt, tag="w2r")
            nc.sync.dma_start(w2r_sb[:], w2r[:])
            b1T_sb = cpool.tile([128, HC], f32, tag="b1T")
            nc.sync.dma_start(b1T_sb[:], b1T[:])
            bias2_sb = cpool.tile([1, 2 * F], mlp_dt, tag="bias2")
            nc.sync.dma_start(bias2_sb[:], bias2[:])

            t0 = 0
            for nts in SUPERS:
                n0 = t0 * 128
                nn = nts * 128
                aggT = aggpool.tile([128, nn], mlp_dt, tag="aggT")
                for st in range(nts):
                    t = t0 + st
                    e_tile = epool.tile([128, c_max, F], edge_dt, tag="e")
                    nc.sync.dma_start(e_tile[:], edges[t])
                    agg_ps = ps_agg.tile([128, 128], f32, tag="agg")
                    for c in range(c_max):
                        oh = ohpool.tile([128, 128], edge_dt, tag="oh")
                        col = t * c_max + c
                        nc.vector.tensor_scalar(
                            out=oh[:],
                            in0=iota_sb[:],
                            scalar1=relT_sb[:, col:col + 1],
                            scalar2=None,
                            op0=mybir.AluOpType.is_equal,
                        )
                        nc.tensor.matmul(
                            agg_ps[:],
                            lhsT=e_tile[:, c, :],
                            rhs=oh[:],
                            start=(c == 0),
                            stop=(c == c_max - 1),
                        )
                    # evacuate psum (cast to mlp dtype)
                    nc.vector.tensor_copy(aggT[:, st * 128:(st + 1) * 128],
                                          agg_ps[:])
                # layer 1 over the supertile, hidden chunk by hidden chunk
                hT = hpool.tile([128, HC, nn], mlp_dt, tag="hT")
                for hc in range(HC):
                    h_ps = ps_h.tile([128, nn], f32, tag="h")
                    nc.tensor.matmul(h_ps[:],
                                     lhsT=w1a_sb[:, hc * 128:(hc + 1) * 128],
                                     rhs=aggT[:],
                                     start=True, stop=False)
                    nc.tensor.matmul(h_ps[:],
                                     lhsT=w1b_sb[:, hc * 128:(hc + 1) * 128],
                                     rhs=xT_sb[:, n0:n0 + nn],
                                     start=False, stop=True)
                    nc.scalar.activation(hT[:, hc, :], h_ps[:],
                                         mybir.ActivationFunctionType.Relu,
                                         bias=b1T_sb[:, hc:hc + 1], scale=1.0)
                # layer 2 per node tile
                for st in range(nts):
                    t = t0 + st
                    o_ps = ps_out.tile([128, F], f32, tag="o")
                    for hc in range(HC):
                        nc.tensor.matmul(
                            o_ps[:],
                            lhsT=hT[:, hc, st * 128:(st + 1) * 128],
                            rhs=w2r_sb[:, hc * 128:(hc + 1) * 128],
                            start=(hc == 0), stop=False)
                    # + b2 as rank-1 outer product ones^T @ b2_row
                    nc.tensor.matmul(o_ps[:],
                                     lhsT=bias2_sb[:1, 0:F],
                                     rhs=bias2_sb[:1, F:2 * F],
                                     start=False, stop=True)
                    o_sb = outpool.tile([128, F], f32, tag="o_sb")
                    nc.vector.tensor_copy(o_sb[:], o_ps[:])
                    nc.sync.dma_start(y[t * 128:(t + 1) * 128, :], o_sb[:])
                t0 += nts

    nc.compile()
    nc.m = get_hw_module(nc.m)
    return nc


# ---------------- host-side sharding / packing ----------------

def prepare_inputs(x, edge_attr, u, W1, b1, W2, b2, edge_index):
    x = np.asarray(x, dtype=np.float32)
    edge_attr = np.asarray(edge_attr, dtype=np.float32)
    u = np.asarray(u, dtype=np.float32)
    W1 = np.asarray(W1, dtype=np.float32)
    b1 = np.asarray(b1, dtype=np.float32)
    W2 = np.asarray(W2, dtype=np.float32)
    b2 = np.asarray(b2, dtype=np.float32)
    recv = np.asarray(edge_index)[1].astype(np.int64)

    edge_np = _np(EDGE_DT)
    mlp_np = _np(MLP_DT)

    # sort edges by receiver; contiguous receiver ranges per core
    order = np.argsort(recv, kind="stable")
    recv_s = recv[order]
    bounds = np.searchsorted(recv_s, np.arange(0, N_NODES + 1, NODES_PC))

    # per-(core,tile) edge counts -> global uniform chunk capacity
    per_core = []
    c_max = 1
    for c in range(N_CORES):
        sl = slice(bounds[c], bounds[c + 1])
        loc = recv_s[sl] - c * NODES_PC
        tid = loc >> 7                       # // 128
        cnt = np.bincount(tid, minlength=NT)
        per_core.append((order[sl], loc, tid, cnt))
        c_max = max(c_max, int(-(-cnt.max() // 128)))

    # shared (replicated) tensors
    b1_eff = b1 + (u[0] @ W1[256:256 + D_U])
    w1a = np.ascontiguousarray(W1[0:128]).astype(mlp_np)
    w1b = np.ascontiguousarray(W1[128:256]).astype(mlp_np)
    w2r = np.ascontiguousarray(
        W2.reshape(H // 128, 128, F).transpose(1, 0, 2).reshape(128, H)
    ).astype(mlp_np)
    b1T = np.ascontiguousarray(
        b1_eff.reshape(H // 128, 128).T).astype(np.float32)
    bias2 = np.concatenate([np.ones(F, np.float32), b2]).reshape(1, 2 * F)
    bias2 = bias2.astype(mlp_np)
    iota = np.tile(np.arange(128, dtype=np.float32), (128, 1)).astype(edge_np)

    in_maps = []
    for c in range(N_CORES):
        perm, loc, tid, cnt = per_core[c]
        off = np.zeros(NT, np.int64)
        np.cumsum(cnt[:-1], out=off[1:])
        s = np.arange(len(perm), dtype=np.int64) - off[tid]   # rank in tile
        p = s & 127
        ch = s >> 7
        slot = (tid * 128 + p) * c_max + ch

        ebuf = np.zeros((NT * 128 * c_max, F), edge_np)
        ebuf[slot] = edge_attr[perm].astype(edge_np)
        ebuf = ebuf.reshape(NT, 128, c_max, F)

        rel = np.full(NT * 128 * c_max, -1.0, np.float32)
        rel[slot] = (loc & 127).astype(np.float32)
        relT = np.ascontiguousarray(
            rel.reshape(NT, 128, c_max).transpose(1, 0, 2).reshape(
                128, NT * c_max)).astype(edge_np)

        xs = x[c * NODES_PC:(c + 1) * NODES_PC]
        xT = np.zeros((128, NODES_PAD), mlp_np)
        xT[:, :NODES_PC] = xs.T.astype(mlp_np)

        in_maps.append({
            "edges": ebuf, "relT": relT, "iota": iota, "xT": xT,
            "w1a": w1a, "w1b": w1b, "w2r": w2r, "b1T": b1T, "bias2": bias2,
        })
    return in_maps, c_max


_prog_cache = {}


def _get_program(c_max):
    key = (c_max, EDGE_DT, MLP_DT)
    if key not in _prog_cache:
        _prog_cache[key] = build_program(c_max, EDGE_DT, MLP_DT)
    return _prog_cache[key]


def run(inputs, trace=False, tmpdir=None):
    in_maps, c_max = prepare_inputs(**inputs)
    nc = _get_program(c_max)
    res = bass_utils.run_bass_kernel_spmd(
        nc, in_maps, core_ids=list(range(N_CORES)), trace=trace,
        tmpdir=tmpdir)
    out = np.concatenate(
        [res.results[c]["y"][:NODES_PC] for c in range(N_CORES)], axis=0)
    return out.astype(np.float32), res


def kernel(**inputs) -> np.ndarray:
    out, _ = run(inputs, trace=False)
    return out


# revision 5
# speedup vs baseline: 1.9189x; 1.9189x over previous
"""GNN NodeBlock kernel for Trainium2, 8 NeuronCores (SPMD, no collectives).

Reference computation (N=50000 nodes, E=1600000 edges, F=128 features):
    recv_agg = segment_sum(edge_attr, edge_index[1], N)        # [N, 128]
    collected = concat([recv_agg, x, broadcast(u)], -1)        # [N, 272]
    out = relu(collected @ W1 + b1) @ W2 + b2                  # [N, 128]

Sharding: nodes are partitioned into 8 contiguous blocks of 6250; edges are
bucketed by receiver-node ownership on the host so the scatter-sum is local
to each core; MLP weights are replicated.  The u-term of layer 1 is constant
across nodes and is folded into b1 on the host (b1_eff = b1 + u @ W1[256:]).

Device algorithm per core (nodes padded to 49 tiles of 128):
  scatter: edges sorted by receiver, grouped per 128-node tile, padded to
    C_max chunks of 128 edges.  For each chunk, DVE builds a one-hot routing
    matrix onehot[e, n] = (rel[e] == n) via tensor_scalar(is_equal) against a
    constant iota tile, and the PE accumulates aggT[f, n] += edge[e, f]^T @
    onehot[e, n] into PSUM.  aggT comes out feature-major — exactly the
    layout layer 1 needs.
  L1: out1T[h, n] = W1a[d, h]^T @ aggT[d, n] + W1b[d, h]^T @ xT[d, n] over
    supertiles of up to 512 nodes, then ReLU+bias on the scalar engine.
  L2: out2[n, f] = sum_h hT[h, n]^T @ W2r[h, f], bias via a rank-1 matmul.
"""

import numpy as np

from concourse import bacc, mybir, tile
from concourse import bass_utils
from concourse.bass_interp import get_hw_module

# ---------------- problem constants (hardcoded per spec) ----------------
N_NODES = 50000
N_EDGES = 1600000
F = 128           # edge/node feature dim
H = 1024          # hidden dim
D_U = 16
N_CORES = 8
NODES_PC = N_NODES // N_CORES          # 6250
NT = (NODES_PC + 127) // 128           # 49 node tiles per core
NODES_PAD = NT * 128                   # 6272
SUPERS = [4] * (NT // 4) + ([NT % 4] if NT % 4 else [])   # [4]*12 + [1]

# dtype knobs: "edge" covers edge data + one-hot; "mlp" covers x/W1/W2 path
EDGE_DT = mybir.dt.bfloat16
MLP_DT = mybir.dt.bfloat16

_np = mybir.dt.np  # mybir dtype -> numpy dtype


# ---------------- device program ----------------

def build_program(c_max: int, edge_dt, mlp_dt):
    f32 = mybir.dt.float32
    nc = bacc.Bacc("TRN2", target_bir_lowering=False, debug=False,
                   num_devices=N_CORES)

    edges = nc.dram_tensor("edges", [NT, 128, c_max, F], edge_dt,
                           kind="ExternalInput").ap()
    # relT must stay fp32: tensor_scalar(is_equal) requires an fp32 scalar AP
    relT = nc.dram_tensor("relT", [128, NT * c_max], f32,
                          kind="ExternalInput").ap()
    iota = nc.dram_tensor("iota", [128, 128], edge_dt,
                          kind="ExternalInput").ap()
    xT = nc.dram_tensor("xT", [128, NODES_PAD], mlp_dt,
                        kind="ExternalInput").ap()
    w1a = nc.dram_tensor("w1a", [128, H], mlp_dt, kind="ExternalInput").ap()
    w1b = nc.dram_tensor("w1b", [128, H], mlp_dt, kind="ExternalInput").ap()
    w2r = nc.dram_tensor("w2r", [128, H], mlp_dt, kind="ExternalInput").ap()
    b1T = nc.dram_tensor("b1T", [128, H // 128], f32,
                         kind="ExternalInput").ap()
    bias2 = nc.dram_tensor("bias2", [1, 2 * F], mlp_dt,
                           kind="ExternalInput").ap()
    y = nc.dram_tensor("y", [NODES_PAD, F], f32, kind="ExternalOutput").ap()

    HC = H // 128  # 8 hidden chunks

    with tile.TileContext(nc) as tc:
        with (
            tc.tile_pool(name="const", bufs=1) as cpool,
            tc.tile_pool(name="edge", bufs=3) as epool,
            tc.tile_pool(name="oh", bufs=6) as ohpool,
            tc.tile_pool(name="agg", bufs=2) as aggpool,
            tc.tile_pool(name="h", bufs=2) as hpool,
            tc.tile_pool(name="out", bufs=3) as outpool,
            tc.tile_pool(name="ps_agg", bufs=2, space="PSUM") as ps_agg,
            tc.tile_pool(name="ps_h", bufs=2, space="PSUM") as ps_h,
            tc.tile_pool(name="ps_out", bufs=2, space="PSUM") as ps_out,
        ):
            # resident constants
            relT_sb = cpool.tile([128, NT * c_max], f32, tag="relT")
            nc.sync.dma_start(relT_sb[:], relT[:])
            iota_sb = cpool.tile([128, 128], edge_dt, tag="iota")
            nc.sync.dma_start(iota_sb[:], iota[:])
            xT_sb = cpool.tile([128, NODES_PAD], mlp_dt, tag="xT")
            nc.sync.dma_start(xT_sb[:], xT[:])
            w1a_sb = cpool.tile([128, H], mlp_dt, tag="w1a")
            nc.sync.dma_start(w1a_sb[:], w1a[:])
            w1b_sb = cpool.tile([128, H], mlp_dt, tag="w1b")
            nc.sync.dma_start(w1b_sb[:], w1b[:])
            w2r_sb = cpool.tile([128, H], mlp_dt, tag="w2r")
            nc.sync.dma_start(w2r_sb[:], w2r[:])
            b1T_sb = cpool.tile([128, HC], f32, tag="b1T")
            nc.sync.dma_start(b1T_sb[:], b1T[:])
            bias2_sb = cpool.tile([1, 2 * F], mlp_dt, tag="bias2")
            nc.sync.dma_start(bias2_sb[:], bias2[:])

            t0 = 0
            for nts in SUPERS:
                n0 = t0 * 128
                nn = nts * 128
                aggT = aggpool.tile([128, nn], mlp_dt, tag="aggT")
                for st in range(nts):
                    t = t0 + st
                    e_tile = epool.tile([128, c_max, F], edge_dt, tag="e")
                    nc.sync.dma_start(e_tile[:], edges[t])
                    agg_ps = ps_agg.tile([128, 128], f32, tag="agg")
                    for c in range(c_max):
                        oh = ohpool.tile([128, 128], edge_dt, tag="oh")
                        col = t * c_max + c
                        nc.vector.tensor_scalar(
                            out=oh[:],
                            in0=iota_sb[:],
                            scalar1=relT_sb[:, col:col + 1],
                            scalar2=None,
                            op0=mybir.AluOpType.is_equal,
                        )
                        nc.tensor.matmul(
                            agg_ps[:],
                            lhsT=e_tile[:, c, :],
                            rhs=oh[:],
                            start=(c == 0),
                            stop=(c == c_max - 1),
                        )
                    # evacuate psum (cast to mlp dtype)
                    nc.vector.tensor_copy(aggT[:, st * 128:(st + 1) * 128],
                                          agg_ps[:])
                # layer 1 over the supertile, hidden chunk by hidden chunk
                hT = hpool.tile([128, HC, nn], mlp_dt, tag="hT")
                for hc in range(HC):
                    h_ps = ps_h.tile([128, nn], f32, tag="h")
                    nc.tensor.matmul(h_ps[:],
                                     lhsT=w1a_sb[:, hc * 128:(hc + 1) * 128],
                                     rhs=aggT[:],
                                     start=True, stop=False)
                    nc.tensor.matmul(h_ps[:],
                                     lhsT=w1b_sb[:, hc * 128:(hc + 1) * 128],
                                     rhs=xT_sb[:, n0:n0 + nn],
                                     start=False, stop=True)
                    nc.scalar.activation(hT[:, hc, :], h_ps[:],
                                         mybir.ActivationFunctionType.Relu,
                                         bias=b1T_sb[:, hc:hc + 1], scale=1.0)
                # layer 2 per node tile
                for st in range(nts):
                    t = t0 + st
                    o_ps = ps_out.tile([128, F], f32, tag="o")
                    for hc in range(HC):
                        nc.tensor.matmul(
                            o_ps[:],
                            lhsT=hT[:, hc, st * 128:(st + 1) * 128],
                            rhs=w2r_sb[:, hc * 128:(hc + 1) * 128],
                            start=(hc == 0), stop=False)
                    # + b2 as rank-1 outer product ones^T @ b2_row
                    nc.tensor.matmul(o_ps[:],
                                     lhsT=bias2_sb[:1, 0:F],
                                     rhs=bias2_sb[:1, F:2 * F],
                                     start=False, stop=True)
                    o_sb = outpool.tile([128, F], f32, tag="o_sb")
                    nc.vector.tensor_copy(o_sb[:], o_ps[:])
                    nc.sync.dma_start(y[t * 128:(t + 1) * 128, :], o_sb[:])
                t0 += nts

    nc.compile()
    nc.m = get_hw_module(nc.m)
    return nc


# ---------------- host-side sharding / packing ----------------

def prepare_inputs(x, edge_attr, u, W1, b1, W2, b2, edge_index):
    x = np.asarray(x, dtype=np.float32)
    edge_attr = np.asarray(edge_attr, dtype=np.float32)
    u = np.asarray(u, dtype=np.float32)
    W1 = np.asarray(W1, dtype=np.float32)
    b1 = np.asarray(b1, dtype=np.float32)
    W2 = np.asarray(W2, dtype=np.float32)
    b2 = np.asarray(b2, dtype=np.float32)
    recv = np.asarray(edge_index)[1].astype(np.int64)

    edge_np = _np(EDGE_DT)
    mlp_np = _np(MLP_DT)

    # sort edges by receiver; contiguous receiver ranges per core
    order = np.argsort(recv, kind="stable")
    recv_s = recv[order]
    bounds = np.searchsorted(recv_s, np.arange(0, N_NODES + 1, NODES_PC))

    # per-(core,tile) edge counts -> global uniform chunk capacity
    per_core = []
    c_max = 1
    for c in range(N_CORES):
        sl = slice(bounds[c], bounds[c + 1])
        loc = recv_s[sl] - c * NODES_PC
        tid = loc >> 7                       # // 128
        cnt = np.bincount(tid, minlength=NT)
        per_core.append((order[sl], loc, tid, cnt))
        c_max = max(c_max, int(-(-cnt.max() // 128)))

    # shared (replicated) tensors
    b1_eff = b1 + (u[0] @ W1[256:256 + D_U])
    w1a = np.ascontiguousarray(W1[0:128]).astype(mlp_np)
    w1b = np.ascontiguousarray(W1[128:256]).astype(mlp_np)
    w2r = np.ascontiguousarray(
        W2.reshape(H // 128, 128, F).transpose(1, 0, 2).reshape(128, H)
    ).astype(mlp_np)
    b1T = np.ascontiguousarray(
        b1_eff.reshape(H // 128, 128).T).astype(np.float32)
    bias2 = np.concatenate([np.ones(F, np.float32), b2]).reshape(1, 2 * F)
    bias2 = bias2.astype(mlp_np)
    iota = np.tile(np.arange(128, dtype=np.float32), (128, 1)).astype(edge_np)

    in_maps = []
    for c in range(N_CORES):
        perm, loc, tid, cnt = per_core[c]
        off = np.zeros(NT, np.int64)
        np.cumsum(cnt[:-1], out=off[1:])
        s = np.arange(len(perm), dtype=np.int64) - off[tid]   # rank in tile
        p = s & 127
        ch = s >> 7
        slot = (tid * 128 + p) * c_max + ch

        ebuf = np.zeros((NT * 128 * c_max, F), edge_np)
        ebuf[slot] = edge_attr[perm].astype(edge_np)
        ebuf = ebuf.reshape(NT, 128, c_max, F)

        rel = np.full(NT * 128 * c_max, -1.0, np.float32)
        rel[slot] = (loc & 127).astype(np.float32)
        relT = np.ascontiguousarray(
            rel.reshape(NT, 128, c_max).transpose(1, 0, 2).reshape(
                128, NT * c_max))

        xs = x[c * NODES_PC:(c + 1) * NODES_PC]
        xT = np.zeros((128, NODES_PAD), mlp_np)
        xT[:, :NODES_PC] = xs.T.astype(mlp_np)

        in_maps.append({
            "edges": ebuf, "relT": relT, "iota": iota, "xT": xT,
            "w1a": w1a, "w1b": w1b, "w2r": w2r, "b1T": b1T, "bias2": bias2,
        })
    return in_maps, c_max


_prog_cache = {}


def _get_program(c_max):
    key = (c_max, EDGE_DT, MLP_DT)
    if key not in _prog_cache:
        _prog_cache[key] = build_program(c_max, EDGE_DT, MLP_DT)
    return _prog_cache[key]


def run(inputs, trace=False, tmpdir=None):
    in_maps, c_max = prepare_inputs(**inputs)
    nc = _get_program(c_max)
    res = bass_utils.run_bass_kernel_spmd(
        nc, in_maps, core_ids=list(range(N_CORES)), trace=trace,
        tmpdir=tmpdir)
    out = np.concatenate(
        [res.results[c]["y"][:NODES_PC] for c in range(N_CORES)], axis=0)
    return out.astype(np.float32), res


def kernel(**inputs) -> np.ndarray:
    out, _ = run(inputs, trace=False)
    return out


# revision 14
# speedup vs baseline: 3.1882x; 1.6614x over previous
"""GNN NodeBlock kernel for Trainium2, 8 NeuronCores (SPMD, no collectives).

Reference computation (N=50000 nodes, E=1600000 edges, F=128 features):
    recv_agg = segment_sum(edge_attr, edge_index[1], N)        # [N, 128]
    collected = concat([recv_agg, x, broadcast(u)], -1)        # [N, 272]
    out = relu(collected @ W1 + b1) @ W2 + b2                  # [N, 128]

Sharding: nodes are partitioned into 8 contiguous blocks of 6250; edges are
bucketed by receiver-node ownership on the host so the scatter-sum is local
to each core; MLP weights are replicated.  The u-term of layer 1 is constant
across nodes and is folded into b1 on the host (b1_eff = b1 + u @ W1[256:]).

Device algorithm per core (nodes padded to 98 tiles of 64):
  scatter: edges sorted by receiver, grouped per 64-node tile, padded to
    C chunks of 128 edges (C = global max, ~18).  One DVE tensor_tensor
    (is_equal) per tile builds all C one-hot routing blocks
    onehot[e, c, n] = (rel[e, c] == n) by comparing a broadcast iota row
    against broadcast per-chunk receiver offsets; the PE accumulates
    aggT[f, n] += edge_chunk[e, f]^T @ onehot[e, c, :] into PSUM.  aggT
    comes out feature-major — exactly the layout layer 1 needs.
  L1: out1T[h, n] = W1a[d, h]^T @ aggT[d, n] + W1b[d, h]^T @ xT[d, n] over
    supertiles of up to 512 nodes, then ReLU+bias on the scalar engine.
  L2: out2[n, f] = sum_h hT[h, n]^T @ W2r[h, f]; b2 is added during the
    PSUM evacuation.
All matmul inputs are bf16 (fp32 PSUM accumulation); one-hots are exact.
"""

import numpy as np

from concourse import bacc, mybir, tile
from concourse import bass_utils
from concourse.bass_interp import get_hw_module

# ---------------- problem constants (hardcoded per spec) ----------------
N_NODES = 50000
N_EDGES = 1600000
F = 128           # edge/node feature dim
H = 1024          # hidden dim
D_U = 16
N_CORES = 8
NODES_PC = N_NODES // N_CORES          # 6250
TN = 64                                # nodes per scatter tile
NT = (NODES_PC + TN - 1) // TN         # 98 scatter tiles per core
NODES_PAD = NT * TN                    # 6272
SUP = 8                                # scatter tiles per supertile (512 nodes)
SUPERS = [SUP] * (NT // SUP) + ([NT % SUP] if NT % SUP else [])  # [8]*12+[2]

EDGE_DT = mybir.dt.bfloat16
MLP_DT = mybir.dt.bfloat16

_np = mybir.dt.np  # mybir dtype -> numpy dtype


# ---------------- device program ----------------

def build_program(c_max: int, edge_dt, mlp_dt):
    f32 = mybir.dt.float32
    NS = len(SUPERS)
    HC = H // 128  # 8 hidden chunks
    nc = bacc.Bacc("TRN2", target_bir_lowering=False, debug=False,
                   num_devices=N_CORES)

    edges = nc.dram_tensor("edges", [NS, 128, SUP, c_max, F], edge_dt,
                           kind="ExternalInput").ap()
    relT = nc.dram_tensor("relT", [128, NT * c_max], edge_dt,
                          kind="ExternalInput").ap()
    iota = nc.dram_tensor("iota", [128, TN], edge_dt,
                          kind="ExternalInput").ap()
    xT = nc.dram_tensor("xT", [128, NODES_PAD], mlp_dt,
                        kind="ExternalInput").ap()
    w1a = nc.dram_tensor("w1a", [128, H], mlp_dt, kind="ExternalInput").ap()
    w1b = nc.dram_tensor("w1b", [128, H], mlp_dt, kind="ExternalInput").ap()
    w2r = nc.dram_tensor("w2r", [128, H], mlp_dt, kind="ExternalInput").ap()
    b1T = nc.dram_tensor("b1T", [128, HC], f32, kind="ExternalInput").ap()
    b2bc = nc.dram_tensor("b2bc", [128, F], f32, kind="ExternalInput").ap()
    y = nc.dram_tensor("y", [NODES_PAD, F], f32, kind="ExternalOutput").ap()
    # [p, g, f] view of y for batched stores (g = 128-node block)
    yr = y.rearrange("(g p) f -> p g f", p=128)

    with tile.TileContext(nc) as tc:
        with (
            tc.tile_pool(name="const", bufs=1) as cpool,
            tc.tile_pool(name="edge", bufs=2) as epool,
            tc.tile_pool(name="oh", bufs=4) as ohpool,
            tc.tile_pool(name="agg", bufs=2) as aggpool,
            tc.tile_pool(name="h", bufs=2) as hpool,
            tc.tile_pool(name="out", bufs=2) as outpool,
            tc.tile_pool(name="ps_agg", bufs=2, space="PSUM") as ps_agg,
            tc.tile_pool(name="ps_h", bufs=2, space="PSUM") as ps_h,
            tc.tile_pool(name="ps_out", bufs=2, space="PSUM") as ps_out,
        ):
            # resident constants
            relT_sb = cpool.tile([128, NT * c_max], edge_dt, tag="relT")
            nc.sync.dma_start(relT_sb[:], relT[:])
            iota_sb = cpool.tile([128, TN], edge_dt, tag="iota")
            nc.sync.dma_start(iota_sb[:], iota[:])
            xT_sb = cpool.tile([128, NODES_PAD], mlp_dt, tag="xT")
            nc.sync.dma_start(xT_sb[:], xT[:])
            w1a_sb = cpool.tile([128, H], mlp_dt, tag="w1a")
            nc.sync.dma_start(w1a_sb[:], w1a[:])
            w1b_sb = cpool.tile([128, H], mlp_dt, tag="w1b")
            nc.sync.dma_start(w1b_sb[:], w1b[:])
            w2r_sb = cpool.tile([128, H], mlp_dt, tag="w2r")
            nc.sync.dma_start(w2r_sb[:], w2r[:])
            b1T_sb = cpool.tile([128, HC], f32, tag="b1T")
            nc.sync.dma_start(b1T_sb[:], b1T[:])
            b2bc_sb = cpool.tile([128, F], f32, tag="b2bc")
            nc.sync.dma_start(b2bc_sb[:], b2bc[:])

            iota_bc = iota_sb[:].rearrange("p (u n) -> p u n", u=1).broadcast_to(
                [128, c_max, TN])

            t0 = 0
            for s, nts in enumerate(SUPERS):
                nn = nts * TN
                e_sup = epool.tile([128, nts, c_max, F], edge_dt, tag="e")
                nc.sync.dma_start(e_sup[:], edges[s, :, :nts])
                aggT = aggpool.tile([128, nn], mlp_dt, tag="aggT")
                for st in range(nts):
                    t = t0 + st
                    # one-hot blocks for all chunks of this tile in one op
                    oh = ohpool.tile([128, c_max, TN], edge_dt, tag="oh")
                    rel_bc = relT_sb[:, t * c_max:(t + 1) * c_max].rearrange(
                        "p (c u) -> p c u", u=1).broadcast_to(
                        [128, c_max, TN])
                    nc.vector.tensor_tensor(out=oh[:], in0=iota_bc,
                                            in1=rel_bc,
                                            op=mybir.AluOpType.is_equal)
                    agg_ps = ps_agg.tile([128, TN], f32, tag="agg")
                    for c in range(c_max):
                        nc.tensor.matmul(
                            agg_ps[:],
                            lhsT=e_sup[:, st, c, :],
                            rhs=oh[:, c, :],
                            start=(c == 0),
                            stop=(c == c_max - 1),
                        )
                    # evacuate psum on the scalar engine (casts to bf16)
                    nc.scalar.copy(aggT[:, st * TN:(st + 1) * TN], agg_ps[:])
                # layer 1 over the supertile, hidden chunk by hidden chunk
                hT = hpool.tile([128, HC, nn], mlp_dt, tag="hT")
                for hc in range(HC):
                    h_ps = ps_h.tile([128, nn], f32, tag="h")
                    nc.tensor.matmul(h_ps[:],
                                     lhsT=w1a_sb[:, hc * 128:(hc + 1) * 128],
                                     rhs=aggT[:],
                                     start=True, stop=False)
                    nc.tensor.matmul(h_ps[:],
                                     lhsT=w1b_sb[:, hc * 128:(hc + 1) * 128],
                                     rhs=xT_sb[:, t0 * TN:t0 * TN + nn],
                                     start=False, stop=True)
                    nc.scalar.activation(hT[:, hc, :], h_ps[:],
                                         mybir.ActivationFunctionType.Relu,
                                         bias=b1T_sb[:, hc:hc + 1], scale=1.0)
                # layer 2 per 128-node block
                ng = nn // 128
                o_sup = outpool.tile([128, ng, F], f32, tag="o")
                for g in range(ng):
                    o_ps = ps_out.tile([128, F], f32, tag="ops")
                    for hc in range(HC):
                        nc.tensor.matmul(
                            o_ps[:],
                            lhsT=hT[:, hc, g * 128:(g + 1) * 128],
                            rhs=w2r_sb[:, hc * 128:(hc + 1) * 128],
                            start=(hc == 0), stop=(hc == HC - 1))
                    # evacuate psum + add b2 in one pass
                    nc.vector.tensor_tensor(out=o_sup[:, g, :], in0=o_ps[:],
                                            in1=b2bc_sb[:],
                                            op=mybir.AluOpType.add)
                g0 = t0 * TN // 128
                nc.scalar.dma_start(yr[:, g0:g0 + ng, :], o_sup[:])
                t0 += nts

    nc.compile()
    nc.m = get_hw_module(nc.m)
    return nc


# ---------------- host-side sharding / packing ----------------

def prepare_inputs(x, edge_attr, u, W1, b1, W2, b2, edge_index):
    x = np.asarray(x, dtype=np.float32)
    edge_attr = np.asarray(edge_attr, dtype=np.float32)
    u = np.asarray(u, dtype=np.float32)
    W1 = np.asarray(W1, dtype=np.float32)
    b1 = np.asarray(b1, dtype=np.float32)
    W2 = np.asarray(W2, dtype=np.float32)
    b2 = np.asarray(b2, dtype=np.float32)
    recv = np.asarray(edge_index)[1].astype(np.int64)

    edge_np = _np(EDGE_DT)
    mlp_np = _np(MLP_DT)

    # sort edges by receiver; contiguous receiver ranges per core
    order = np.argsort(recv, kind="stable")
    recv_s = recv[order]
    bounds = np.searchsorted(recv_s, np.arange(0, N_NODES + 1, NODES_PC))

    # per-(core,tile) edge counts -> global uniform chunk capacity
    per_core = []
    c_max = 1
    for c in range(N_CORES):
        sl = slice(bounds[c], bounds[c + 1])
        loc = recv_s[sl] - c * NODES_PC
        tid = loc // TN
        cnt = np.bincount(tid, minlength=NT)
        per_core.append((order[sl], loc, tid, cnt))
        c_max = max(c_max, int(-(-cnt.max() // 128)))

    # shared (replicated) tensors
    b1_eff = b1 + (u[0] @ W1[256:256 + D_U])
    w1a = np.ascontiguousarray(W1[0:128]).astype(mlp_np)
    w1b = np.ascontiguousarray(W1[128:256]).astype(mlp_np)
    w2r = np.ascontiguousarray(
        W2.reshape(H // 128, 128, F).transpose(1, 0, 2).reshape(128, H)
    ).astype(mlp_np)
    b1T = np.ascontiguousarray(
        b1_eff.reshape(H // 128, 128).T).astype(np.float32)
    b2bc = np.ascontiguousarray(np.tile(b2, (128, 1))).astype(np.float32)
    iota = np.tile(np.arange(TN, dtype=np.float32), (128, 1)).astype(edge_np)

    NS = len(SUPERS)
    in_maps = []
    for c in range(N_CORES):
        perm, loc, tid, cnt = per_core[c]
        off = np.zeros(NT, np.int64)
        np.cumsum(cnt[:-1], out=off[1:])
        s = np.arange(len(perm), dtype=np.int64) - off[tid]   # rank in tile
        p = s & 127
        ch = s >> 7
        sup = tid // SUP
        st = tid % SUP
        slot = ((sup * 128 + p) * SUP + st) * c_max + ch

        ebuf = np.zeros((NS * 128 * SUP * c_max, F), edge_np)
        ebuf[slot] = edge_attr[perm].astype(edge_np)
        ebuf = ebuf.reshape(NS, 128, SUP, c_max, F)

        rel = np.full(NT * 128 * c_max, -1.0, np.float32)
        rslot = (tid * 128 + p) * c_max + ch
        rel[rslot] = (loc - tid * TN).astype(np.float32)
        relT = np.ascontiguousarray(
            rel.reshape(NT, 128, c_max).transpose(1, 0, 2).reshape(
                128, NT * c_max)).astype(edge_np)

        xs = x[c * NODES_PC:(c + 1) * NODES_PC]
        xT = np.zeros((128, NODES_PAD), mlp_np)
        xT[:, :NODES_PC] = xs.T.astype(mlp_np)

        in_maps.append({
            "edges": ebuf, "relT": relT, "iota": iota, "xT": xT,
            "w1a": w1a, "w1b": w1b, "w2r": w2r, "b1T": b1T, "b2bc": b2bc,
        })
    return in_maps, c_max


_prog_cache = {}


def _get_program(c_max):
    key = (c_max, EDGE_DT, MLP_DT)
    if key not in _prog_cache:
        _prog_cache[key] = build_program(c_max, EDGE_DT, MLP_DT)
    return _prog_cache[key]


def run(inputs, trace=False, tmpdir=None):
    in_maps, c_max = prepare_inputs(**inputs)
    nc = _get_program(c_max)
    res = bass_utils.run_bass_kernel_spmd(
        nc, in_maps, core_ids=list(range(N_CORES)), trace=trace,
        tmpdir=tmpdir)
    out = np.concatenate(
        [res.results[c]["y"][:NODES_PC] for c in range(N_CORES)], axis=0)
    return out.astype(np.float32), res


def kernel(**inputs) -> np.ndarray:
    out, _ = run(inputs, trace=False)
    return out
